# revision 5
# baseline (speedup 1.0000x reference)
"""Trainium2 Bass kernel for nn_AttentionBlock (GroupNorm + single-head
self-attention + proj + residual), data-parallel over batch on 8 cores.

Contract: kernel(**inputs) takes the FULL unsharded inputs
  x (8, 256, 64, 64) f32, gn_scale (256,), gn_bias (256,),
  qkv_w (768, 256), qkv_b (768,), proj_w (256, 256), proj_b (256,)
and returns the FULL output (8, 256, 64, 64) f32.

v2 design (from the v1 NTFF trace: PE 90% busy on matmuls, ACT co-bound
on exp, DVE saturated by denominator accumulation):
  - GroupNorm folded into the QKV weights: w_eff[c,o] = 32*W[o,c]*m_c on
    device (m_c = rstd*gamma per channel), so no xn tensor is ever
    materialized. The additive GN term (a_c) becomes per-output biases
    via tiny matmuls (W@a). x is shipped from host in BOTH f32 (GN stats
    + residual) and fp8 (QKV matmul operand).
  - QKV/scores/PV all fp8 DoubleRow (K=256 in one pass).
  - Softmax denominator on the PE: a ones-lhsT DR matmul per key-block
    pair accumulates den[q] into the same PSUM tile group as the PV
    output (tile [P, 3, 512]: ch0, ch1, den) -> zero DVE work in the
    steady loop.
  - Steady state per 512-q-tile step: PE 5 matmuls (2 scores, 2 PV,
    1 den) ~1.1us; ACT one 1024-wide exp ~1.1us. PV/den run one step
    behind scores so ACT never waits on PE.
  - PSUM banks: scores 2x[P,2,512]=4, out+den [P,3,512]=3, proj 1 = 8.
"""

import os
import sys

import numpy as np

for _p in (
    "/opt/trn_rl_repo",
    "/root/.axon_site",
    "/root/.axon_site/_ro/trn_rl_repo",
    "/root/.axon_site/_ro/pypackages",
):
    if os.path.isdir(_p) and _p not in sys.path:
        sys.path.append(_p)

import ml_dtypes  # noqa: E402

import concourse.bass as bass  # noqa: E402
import concourse.mybir as mybir  # noqa: E402
import concourse.tile as tile  # noqa: E402
from concourse import bacc  # noqa: E402

F32 = mybir.dt.float32
BF16 = mybir.dt.bfloat16
FP8 = mybir.dt.float8e4
AF = mybir.ActivationFunctionType
ALU = mybir.AluOpType
DR = mybir.MatmulPerfMode.DoubleRow

B, C, H, W = 8, 256, 64, 64
GROUPS = 8
EPS = 1e-5
P = 128
N_CORES = 8
ATT_SCALE = float(C) ** -0.5  # 1/16
WS = 32.0                     # host pre-scale on fp8 qkv weights
INV_WS = 1.0 / WS


def build_nc(n_tok=H * W):
    """Build the single-core Bass program (SPMD across 8 cores)."""
    CCH = C // P            # channel chunks (2)
    QT = 512                # q-tile width (one PSUM bank of f32)
    NQ = n_tok // QT        # number of q tiles (8)
    NKB = n_tok // P        # number of 128-token key blocks (32)
    NKP = NKB // 2          # key-block pairs per q tile (16)
    GSZ = C // GROUPS       # channels per group (32)

    nc = bacc.Bacc()

    # ---- DRAM I/O (per-core tensors; host shards batch over cores) ----
    x_d = nc.dram_tensor("x", [C, n_tok], F32, kind="ExternalInput")
    xf8_d = nc.dram_tensor("x_f8", [CCH, P, n_tok], FP8, kind="ExternalInput")
    qkvw_d = nc.dram_tensor("qkv_wt", [CCH, P, 3 * C], FP8, kind="ExternalInput")
    qkbias_d = nc.dram_tensor("qk_bias", [4, P, 1], F32, kind="ExternalInput")
    vbias_d = nc.dram_tensor("v_bias", [C], F32, kind="ExternalInput")
    projw_d = nc.dram_tensor("proj_wt", [CCH, P, C], BF16, kind="ExternalInput")
    projb_d = nc.dram_tensor("proj_b", [CCH, P, 1], F32, kind="ExternalInput")
    gnsc_d = nc.dram_tensor("gn_sc", [CCH, P, 1], F32, kind="ExternalInput")
    gnbi_d = nc.dram_tensor("gn_bi", [CCH, P, 1], F32, kind="ExternalInput")
    # group-sum indicator (zero-padded to M=128): ind[t, c, g] = (t*128+c)//32 == g
    gnind_d = nc.dram_tensor("gn_ind", [CCH, P, P], F32, kind="ExternalInput")
    # channel-broadcast indicator, padded to K=128: ind2[t, g, c] nonzero only g<8
    gnind2_d = nc.dram_tensor("gn_ind2", [CCH, P, P], F32, kind="ExternalInput")
    out_d = nc.dram_tensor("out", [C, n_tok], F32, kind="ExternalOutput")

    with tile.TileContext(nc) as tc:
        with (
            tc.tile_pool(name="persist", bufs=1) as pp,
            tc.tile_pool(name="work", bufs=3) as wp,
            tc.tile_pool(name="ps", bufs=1, space="PSUM") as psp,
        ):
            # ---------------- load weights / constants ----------------
            qkvw = pp.tile([P, CCH, 3 * C], FP8, tag="qkvw")
            nc.sync.dma_start(qkvw[:], qkvw_d.rearrange("t p o -> p t o"))
            projw = pp.tile([P, CCH, C], BF16, tag="projw")
            nc.sync.dma_start(projw[:], projw_d.rearrange("t p o -> p t o"))
            qkb = pp.tile([P, 4], F32, tag="qkb")
            nc.sync.dma_start(qkb[:], qkbias_d.rearrange("j p one -> p (j one)"))
            projb = pp.tile([P, CCH], F32, tag="projb")
            nc.sync.dma_start(projb[:], projb_d.rearrange("t p one -> p (t one)"))
            gnsc = pp.tile([P, CCH], F32, tag="gnsc")
            nc.sync.dma_start(gnsc[:], gnsc_d.rearrange("t p one -> p (t one)"))
            gnbi = pp.tile([P, CCH], F32, tag="gnbi")
            nc.sync.dma_start(gnbi[:], gnbi_d.rearrange("t p one -> p (t one)"))
            gnind = pp.tile([P, CCH, P], F32, tag="gnind")
            nc.sync.dma_start(gnind[:], gnind_d.rearrange("t p g -> p t g"))
            gnind2 = pp.tile([P, CCH, P], F32, tag="gnind2")
            nc.sync.dma_start(gnind2[:], gnind2_d.rearrange("t g c -> g t c"))
            # V bias broadcast across partitions (DMA with partition-stride 0)
            vbias = pp.tile([P, C], F32, tag="vbias")
            nc.sync.dma_start(vbias[:], vbias_d[None, :].to_broadcast([P, C]))
            # ones for the denominator matmul (fp8, DR: [K=128, 2, M=128])
            ones_f8 = pp.tile([P, 2, P], FP8, tag="ones_f8")
            nc.vector.memset(ones_f8[:], 1.0)
            # single-partition ones column for the V-bias broadcast matmul
            onescol = pp.tile([1, P], F32, tag="onescol")
            nc.vector.memset(onescol[:], 1.0)

            # ---------------- load x, GroupNorm stats ----------------
            x_sb = pp.tile([P, CCH, n_tok], F32, tag="x_sb")
            stats = pp.tile([P, CCH, 2], F32, tag="stats")
            XPC = max(1, n_tok // 1024)
            for t in range(CCH):
                for pc in range(XPC):
                    xs = slice(pc * (n_tok // XPC), (pc + 1) * (n_tok // XPC))
                    nc.sync.dma_start(x_sb[:, t, xs], x_d[t * P:(t + 1) * P, xs])
                bn6 = wp.tile([P, n_tok // 512, 6], F32, tag="bn6")
                xv = x_sb[:, t].rearrange("p (a b) -> p a b", b=512)
                for a in range(n_tok // 512):
                    nc.vector.bn_stats(bn6[:, a], xv[:, a])
                nc.vector.bn_aggr(stats[:, t], bn6[:])
                # stats col1 := mean^2 + var = E[x^2] (col0 stays mean)
                nc.vector.scalar_tensor_tensor(
                    out=stats[:, t, 1:2],
                    in0=stats[:, t, 0:1],
                    scalar=stats[:, t, 0:1],
                    in1=stats[:, t, 1:2],
                    op0=ALU.mult,
                    op1=ALU.add,
                )
            # x in fp8 for the QKV matmuls (host precast)
            x_f8 = pp.tile([P, CCH, n_tok], FP8, tag="x_f8")
            nc.sync.dma_start(x_f8[:], xf8_d.rearrange("t p n -> p t n"))

            # group aggregation: gagg[g, j] = sum_{c in group g} stats[c, j]
            gagg_ps = psp.tile([P, QT], F32, tag="p", name="gagg_ps")
            for t in range(CCH):
                nc.tensor.matmul(
                    gagg_ps[:, :2],
                    gnind[:, t],
                    stats[:, t],
                    start=(t == 0),
                    stop=(t == CCH - 1),
                )
            # per-group a = rstd, b = -mean * rstd
            gab = pp.tile([P, 2], F32, tag="gab")
            nc.vector.memset(gab[:], 0.0)
            gmean = wp.tile([P, 1], F32, tag="gmean")
            gtmp = wp.tile([P, 1], F32, tag="gtmp")
            nc.vector.tensor_scalar_mul(gmean[:GROUPS], gagg_ps[:GROUPS, 0:1], 1.0 / GSZ)
            nc.vector.tensor_scalar_mul(gtmp[:GROUPS], gagg_ps[:GROUPS, 1:2], 1.0 / GSZ)
            # gtmp := mean^2 - E[x^2] = -var
            nc.vector.scalar_tensor_tensor(
                out=gtmp[:GROUPS],
                in0=gmean[:GROUPS],
                scalar=gmean[:GROUPS],
                in1=gtmp[:GROUPS],
                op0=ALU.mult,
                op1=ALU.subtract,
            )
            # std = sqrt(-1 * gtmp + eps)
            epsb = wp.tile([P, 1], F32, tag="epsb")
            nc.vector.memset(epsb[:], EPS)
            nc.scalar.activation(gtmp[:GROUPS], gtmp[:GROUPS], AF.Sqrt,
                                 bias=epsb[:GROUPS], scale=-1.0)
            nc.vector.reciprocal(gab[:GROUPS, 0:1], gtmp[:GROUPS])  # a = rstd
            nc.vector.tensor_mul(gtmp[:GROUPS], gmean[:GROUPS], gab[:GROUPS, 0:1])
            nc.vector.tensor_scalar_mul(gab[:GROUPS, 1:2], gtmp[:GROUPS], -1.0)

            # broadcast (a, b) to channels; fold GN into the fp8 weights:
            #   m_c = rstd_g * gamma_c ; a_c = (-mean*rstd)*gamma_c + beta_c
            #   w_eff[c, o] = qkvw[c, o] * m_c        (qkvw = 32*W^T)
            #   a2_c = 32 * a_c / m_c   (fp8; a-term via w_eff @ a2 / 1024)
            w_eff = pp.tile([P, CCH, 3 * C], FP8, tag="w_eff")
            a_col = pp.tile([P, CCH, 1], FP8, tag="a_col")
            chms = []
            for t in range(CCH):
                chab_ps = psp.tile([P, QT], F32, tag="p", name=f"chab_ps{t}")
                nc.tensor.matmul(chab_ps[:, :2], gnind2[:, t], gab[:],
                                 start=True, stop=True)
                chm = pp.tile([P, 1], F32, tag=f"chm{t}", name=f"chm{t}")
                cha = pp.tile([P, 1], F32, tag=f"cha{t}", name=f"cha{t}")
                nc.vector.tensor_mul(chm[:], chab_ps[:, 0:1], gnsc[:, t, None])
                nc.vector.scalar_tensor_tensor(
                    out=cha[:],
                    in0=chab_ps[:, 1:2],
                    scalar=gnsc[:, t, None],
                    in1=gnbi[:, t, None],
                    op0=ALU.mult,
                    op1=ALU.add,
                )
                chms.append(chm)
                nc.vector.tensor_scalar_mul(w_eff[:, t], qkvw[:, t], chm[:])
                # a2 = 32 * cha / chm  (fp8)
                rchm = wp.tile([P, 1], F32, tag=f"rchm{t}", name=f"rchm{t}")
                nc.vector.reciprocal(rchm[:], chm[:])
                nc.vector.scalar_tensor_tensor(
                    out=a_col[:, t],
                    in0=cha[:],
                    scalar=WS,
                    in1=rchm[:],
                    op0=ALU.mult,
                    op1=ALU.mult,
                )

            # ---- bias vectors: Wa terms via tiny matmuls ----
            # Q/K: qkb_eff[d, j] = qkb[d, j] + (W@a)[j*128+d] / 1024
            qkb_eff = pp.tile([P, 4], F32, tag="qkb_eff")
            for j in range(4):
                wa_ps = psp.tile([P, QT], F32, tag="p", name=f"wa_ps{j}")
                for t in range(CCH):
                    nc.tensor.matmul(
                        wa_ps[:, 0:1],
                        w_eff[:, t, j * P:(j + 1) * P],
                        a_col[:, t],
                        start=(t == 0),
                        stop=(t == CCH - 1),
                    )
                nc.vector.scalar_tensor_tensor(
                    out=qkb_eff[:, j, None],
                    in0=wa_ps[:, 0:1],
                    scalar=1.0 / (WS * WS),
                    in1=qkb[:, j, None],
                    op0=ALU.mult,
                    op1=ALU.add,
                )
            # V: vb_eff[*, d] = vbias[d] + (Wv@a)[d] / 1024, broadcast to all
            # partitions via a ones-column fp32 matmul.
            vr_ps = psp.tile([P, QT], F32, tag="p", name="vr_ps")
            for t in range(CCH):
                nc.tensor.matmul(
                    vr_ps[:1, :C],
                    a_col[:, t],
                    w_eff[:, t, 2 * C:3 * C],
                    start=(t == 0),
                    stop=(t == CCH - 1),
                )
            vrow = pp.tile([1, C], F32, tag="vrow")
            nc.vector.tensor_copy(vrow[:], vr_ps[:1, :C])
            vb_ps = psp.tile([P, QT], F32, tag="p", name="vb_ps")
            nc.tensor.matmul(vb_ps[:, :C], onescol[:], vrow[:],
                             start=True, stop=True)
            vb_eff = pp.tile([P, C], F32, tag="vb_eff")
            nc.vector.scalar_tensor_tensor(
                out=vb_eff[:],
                in0=vb_ps[:, :C],
                scalar=1.0 / (WS * WS),
                in1=vbias[:],
                op0=ALU.mult,
                op1=ALU.add,
            )

            # ---------------- QKV ----------------
            # Q, K in (d, n) fp8; V token-major fp8. All matmuls fp8 DR
            # (K=256 contraction in one pass). Copies spread across ACT
            # (Q), DVE (K), and GpSimd (V) so no engine serializes the
            # prologue. Interleave emission so the three copy engines run
            # concurrently under the bufs=2 "s" psum rotation.
            qk = pp.tile([P, 4, n_tok], FP8, tag="qk")
            v_sb = pp.tile([P, NKB, C], FP8, tag="v_sb")

            def emit_qk_block(j, nb):
                # one [P, 1024] block of Q (j<2) or K (j>=2)
                ns = slice(nb * 1024, (nb + 1) * 1024)
                qp = psp.tile([P, 2, QT], F32, tag="s", bufs=2,
                              name=f"qp{j}_{nb}")
                for half in range(2):
                    nsh = slice(nb * 1024 + half * QT, nb * 1024 + (half + 1) * QT)
                    nc.tensor.matmul(
                        qp[:, half],
                        w_eff[:, :, j * P:(j + 1) * P],
                        x_f8[:, :, nsh],
                        start=True,
                        stop=True,
                        perf_mode=DR,
                    )
                if j < 3:
                    nc.scalar.activation(
                        qk[:, j, ns], qp.rearrange("p a b -> p (a b)"),
                        AF.Identity,
                        bias=qkb_eff[:, j, None],
                        scale=INV_WS,
                    )
                else:
                    nc.vector.tensor_scalar(
                        out=qk[:, j, ns],
                        in0=qp.rearrange("p a b -> p (a b)"),
                        scalar1=INV_WS,
                        scalar2=qkb_eff[:, j, None],
                        op0=ALU.mult,
                        op1=ALU.add,
                    )

            def emit_v_block(tp):
                # two 128-token blocks of V: out [tok, 2, C]
                vp = psp.tile([P, 2, C], F32, tag="s", bufs=2, name=f"vp{tp}")
                for k2 in range(2):
                    tb = 2 * tp + k2
                    nc.tensor.matmul(
                        vp[:, k2],
                        x_f8[:, :, tb * P:(tb + 1) * P],
                        w_eff[:, :, 2 * C:3 * C],
                        start=True,
                        stop=True,
                        perf_mode=DR,
                    )
                nc.vector.scalar_tensor_tensor(
                    out=v_sb[:, 2 * tp:2 * tp + 2],
                    in0=vp[:],
                    scalar=INV_WS,
                    in1=vb_eff[:, None, :].to_broadcast([P, 2, C]),
                    op0=ALU.mult,
                    op1=ALU.add,
                )

            # rounds: V first (slowest copies), then Q (ACT), then K (DVE)
            for r in range(8):
                emit_v_block(2 * r)
                emit_qk_block(r % 2, r // 2)          # Q blocks
                emit_v_block(2 * r + 1)
                emit_qk_block(2 + r % 2, r // 2)      # K blocks

            # ---------------- attention ----------------
            # global steps g = qt*NKP + i ; per step:
            #   scores(g):  2 DR matmuls -> s_ps [P, 2(kb), 512]
            #   exp(g):     1 ACT instr [P, 1024] -> pt fp8
            #   pv_den(g-1): 2 PV DR matmuls + 1 ones-DR matmul into
            #                o tile [P, 3, 512] (ch0, ch1, den)
            # finalize(qt-1) emitted at (qt, i==1), before pv_den(qt, 0),
            # so the tile framework orders the o-tile reuse correctly.
            o_tiles = {}
            pt_tiles = {}

            def emit_scores_exp(g):
                qt, i = divmod(g, NKP)
                qs = slice(qt * QT, (qt + 1) * QT)
                s_ps = psp.tile([P, 2, QT], F32, tag="s", bufs=2,
                                name=f"s_{g}")
                for k2 in range(2):
                    kb = 2 * i + k2
                    nc.tensor.matmul(
                        s_ps[:, k2],
                        qk[:, 2:4, kb * P:(kb + 1) * P],
                        qk[:, 0:2, qs],
                        start=True,
                        stop=True,
                        perf_mode=DR,
                    )
                pt = wp.tile([P, 2, QT], FP8, tag="pt", bufs=4, name=f"pt_{g}")
                nc.scalar.activation(
                    pt.rearrange("p a b -> p (a b)"),
                    s_ps.rearrange("p a b -> p (a b)"),
                    AF.Exp, scale=ATT_SCALE)
                pt_tiles[g] = pt

            def emit_pv_den(g):
                qt, i = divmod(g, NKP)
                if i == 0:
                    o_tiles[qt] = psp.tile([P, 3, QT], F32, tag="o",
                                           name=f"o_{qt}")
                o = o_tiles[qt]
                pt = pt_tiles.pop(g)
                for ch in range(2):
                    nc.tensor.matmul(
                        o[:, ch],
                        v_sb[:, 2 * i:2 * i + 2, ch * P:(ch + 1) * P],
                        pt[:],
                        start=(i == 0),
                        stop=(i == NKP - 1),
                        perf_mode=DR,
                    )
                nc.tensor.matmul(
                    o[:, 2],
                    ones_f8[:],
                    pt[:],
                    start=(i == 0),
                    stop=(i == NKP - 1),
                    perf_mode=DR,
                )

            def finalize(qt):
                qs = slice(qt * QT, (qt + 1) * QT)
                o = o_tiles.pop(qt)
                rec = wp.tile([P, QT], F32, tag="rec", bufs=2, name=f"rec{qt}")
                nc.vector.reciprocal_approx_fast(rec[:], o[:, 2])
                obs = wp.tile([P, 2, QT], BF16, tag="obs", bufs=2,
                              name=f"obs{qt}")
                for ch in range(2):
                    nc.vector.tensor_mul(obs[:, ch], o[:, ch], rec[:])
                for t in range(CCH):
                    p_ps = psp.tile([P, QT], F32, tag="p", name=f"pp_{qt}_{t}")
                    nc.tensor.matmul(p_ps[:],
                                     projw[:, 0, t * P:(t + 1) * P],
                                     obs[:, 0], start=True, stop=False)
                    nc.tensor.matmul(p_ps[:],
                                     projw[:, 1, t * P:(t + 1) * P],
                                     obs[:, 1], start=False, stop=True)
                    res = wp.tile([P, QT], F32, tag="res", bufs=4,
                                  name=f"res{qt}_{t}")
                    nc.vector.scalar_tensor_tensor(
                        out=res[:],
                        in0=p_ps[:],
                        scalar=projb[:, t, None],
                        in1=x_sb[:, t, qs],
                        op0=ALU.add,
                        op1=ALU.add,
                    )
                    nc.sync.dma_start(out_d[t * P:(t + 1) * P, qs], res[:])

            NST = NQ * NKP
            for g in range(NST):
                qt, i = divmod(g, NKP)
                emit_scores_exp(g)
                if qt > 0 and i == 1:
                    finalize(qt - 1)
                if g > 0:
                    emit_pv_den(g - 1)
            emit_pv_den(NST - 1)
            finalize(NQ - 1)

    nc.finalize()
    return nc


# ---------------------------------------------------------------------------
# host side
# ---------------------------------------------------------------------------

def _prep_core_inputs(inputs, n_tok=H * W):
    """Build the per-core in_maps (shared weight tensors + per-core x)."""
    CCH = C // P
    f32 = np.float32
    bf16 = ml_dtypes.bfloat16
    fp8 = mybir.dt.np(FP8)

    x = np.asarray(inputs["x"], f32).reshape(B, C, n_tok)
    gn_scale = np.asarray(inputs["gn_scale"], f32)
    gn_bias = np.asarray(inputs["gn_bias"], f32)
    qkv_w = np.asarray(inputs["qkv_w"], f32)
    qkv_b = np.asarray(inputs["qkv_b"], f32)
    proj_w = np.asarray(inputs["proj_w"], f32)
    proj_b = np.asarray(inputs["proj_b"], f32)

    qkv_wt = (np.ascontiguousarray(qkv_w.T) * WS).reshape(CCH, P, 3 * C).astype(fp8)
    qk_bias = qkv_b[:2 * C].reshape(4, P, 1).astype(f32).copy()
    v_bias = qkv_b[2 * C:].astype(f32)
    proj_wt = np.ascontiguousarray(proj_w.T).reshape(CCH, P, C).astype(bf16)
    proj_bt = proj_b.reshape(CCH, P, 1).astype(f32)
    gn_sc = gn_scale.reshape(CCH, P, 1).astype(f32)
    gn_bi = gn_bias.reshape(CCH, P, 1).astype(f32)

    ch = np.arange(C)
    gn_ind = np.zeros((CCH, P, P), f32)
    gn_ind[ch // P, ch % P, ch // (C // GROUPS)] = 1.0
    gn_ind2 = np.zeros((CCH, P, P), f32)
    for t in range(CCH):
        gn_ind2[t, :GROUPS, :] = gn_ind[t, :, :GROUPS].T

    shared = {
        "qkv_wt": qkv_wt,
        "qk_bias": qk_bias,
        "v_bias": v_bias,
        "proj_wt": proj_wt,
        "proj_b": proj_bt,
        "gn_sc": gn_sc,
        "gn_bi": gn_bi,
        "gn_ind": gn_ind,
        "gn_ind2": gn_ind2,
    }
    x_f8 = x.reshape(B, CCH, P, n_tok).astype(fp8)
    return [
        dict(shared, x=np.ascontiguousarray(x[i]), x_f8=np.ascontiguousarray(x_f8[i]))
        for i in range(B)
    ]


_NC_CACHE = {}
LAST_RESULT = None  # BassKernelResults of the most recent run (for test.py)


def _get_nc():
    if "nc" not in _NC_CACHE:
        _NC_CACHE["nc"] = build_nc()
    return _NC_CACHE["nc"]


def kernel(**inputs) -> np.ndarray:
    global LAST_RESULT
    from concourse.bass_utils import run_bass_kernel_spmd

    nc = _get_nc()
    in_maps = _prep_core_inputs(inputs)
    res = run_bass_kernel_spmd(nc, in_maps, list(range(N_CORES)))
    LAST_RESULT = res
    out = np.stack([np.asarray(res.results[i]["out"]) for i in range(B)])
    return out.reshape(B, C, H, W).astype(np.float32)


# revision 12
# speedup vs baseline: 1.1287x; 1.1287x over previous
"""Trainium2 Bass kernel for nn_AttentionBlock (GroupNorm + single-head
self-attention + proj + residual), data-parallel over batch on 8 cores.

Contract: kernel(**inputs) takes the FULL unsharded inputs
  x (8, 256, 64, 64) f32, gn_scale (256,), gn_bias (256,),
  qkv_w (768, 256), qkv_b (768,), proj_w (256, 256), proj_b (256,)
and returns the FULL output (8, 256, 64, 64) f32.

v2 design (from the v1 NTFF trace: PE 90% busy on matmuls, ACT co-bound
on exp, DVE saturated by denominator accumulation):
  - GroupNorm folded into the QKV weights: w_eff[c,o] = 32*W[o,c]*m_c on
    device (m_c = rstd*gamma per channel), so no xn tensor is ever
    materialized. The additive GN term (a_c) becomes per-output biases
    via tiny matmuls (W@a). x is shipped from host in BOTH f32 (GN stats
    + residual) and fp8 (QKV matmul operand).
  - QKV/scores/PV all fp8 DoubleRow (K=256 in one pass).
  - Softmax denominator on the PE: a ones-lhsT DR matmul per key-block
    pair accumulates den[q] into the same PSUM tile group as the PV
    output (tile [P, 3, 512]: ch0, ch1, den) -> zero DVE work in the
    steady loop.
  - Steady state per 512-q-tile step: PE 5 matmuls (2 scores, 2 PV,
    1 den) ~1.1us; ACT one 1024-wide exp ~1.1us. PV/den run one step
    behind scores so ACT never waits on PE.
  - PSUM banks: scores 2x[P,2,512]=4, out+den [P,3,512]=3, proj 1 = 8.
"""

import os
import sys

import numpy as np

for _p in (
    "/opt/trn_rl_repo",
    "/root/.axon_site",
    "/root/.axon_site/_ro/trn_rl_repo",
    "/root/.axon_site/_ro/pypackages",
):
    if os.path.isdir(_p) and _p not in sys.path:
        sys.path.append(_p)

import ml_dtypes  # noqa: E402

import concourse.bass as bass  # noqa: E402
import concourse.mybir as mybir  # noqa: E402
import concourse.tile as tile  # noqa: E402
from concourse import bacc  # noqa: E402

F32 = mybir.dt.float32
BF16 = mybir.dt.bfloat16
FP8 = mybir.dt.float8e4
AF = mybir.ActivationFunctionType
ALU = mybir.AluOpType
DR = mybir.MatmulPerfMode.DoubleRow

B, C, H, W = 8, 256, 64, 64
GROUPS = 8
EPS = 1e-5
P = 128
N_CORES = 8
ATT_SCALE = float(C) ** -0.5  # 1/16
WS = 32.0                     # host pre-scale on fp8 qkv weights
INV_WS = 1.0 / WS


def build_nc(n_tok=H * W):
    """Build the single-core Bass program (SPMD across 8 cores)."""
    CCH = C // P            # channel chunks (2)
    QT = 512                # q-tile width (one PSUM bank of f32)
    NQ = n_tok // QT        # number of q tiles (8)
    NKB = n_tok // P        # number of 128-token key blocks (32)
    NKP = NKB // 2          # key-block pairs per q tile (16)
    GSZ = C // GROUPS       # channels per group (32)

    nc = bacc.Bacc()

    # ---- DRAM I/O (per-core tensors; host shards batch over cores) ----
    x_d = nc.dram_tensor("x", [C, n_tok], F32, kind="ExternalInput")
    xf8_d = nc.dram_tensor("x_f8", [CCH, P, n_tok], FP8, kind="ExternalInput")
    qkvw_d = nc.dram_tensor("qkv_wt", [CCH, P, 3 * C], FP8, kind="ExternalInput")
    qkbias_d = nc.dram_tensor("qk_bias", [4, P, 1], F32, kind="ExternalInput")
    vbias_d = nc.dram_tensor("v_bias", [C], F32, kind="ExternalInput")
    projw_d = nc.dram_tensor("proj_wt", [CCH, P, C], BF16, kind="ExternalInput")
    projb_d = nc.dram_tensor("proj_b", [CCH, P, 1], F32, kind="ExternalInput")
    gnsc_d = nc.dram_tensor("gn_sc", [CCH, P, 1], F32, kind="ExternalInput")
    gnbi_d = nc.dram_tensor("gn_bi", [CCH, P, 1], F32, kind="ExternalInput")
    # group-sum indicator (zero-padded to M=128): ind[t, c, g] = (t*128+c)//32 == g
    gnind_d = nc.dram_tensor("gn_ind", [CCH, P, P], F32, kind="ExternalInput")
    # channel-broadcast indicator, padded to K=128: ind2[t, g, c] nonzero only g<8
    gnind2_d = nc.dram_tensor("gn_ind2", [CCH, P, P], F32, kind="ExternalInput")
    out_d = nc.dram_tensor("out", [C, n_tok], F32, kind="ExternalOutput")

    with tile.TileContext(nc) as tc:
        with (
            tc.tile_pool(name="persist", bufs=1) as pp,
            tc.tile_pool(name="work", bufs=3) as wp,
            tc.tile_pool(name="ps", bufs=1, space="PSUM") as psp,
        ):
            # ---------------- load x_f8 first (feeds GN stats + QKV) ------
            x_f8 = pp.tile([P, CCH, n_tok], FP8, tag="x_f8")
            XPC = 4
            for pc in range(XPC):
                xs = slice(pc * (n_tok // XPC), (pc + 1) * (n_tok // XPC))
                nc.sync.dma_start(
                    x_f8[:, :, xs],
                    xf8_d.rearrange("t p n -> p t n")[:, :, xs],
                )

            # ---------------- load weights / constants ----------------
            qkvw = pp.tile([P, CCH, 3 * C], FP8, tag="qkvw")
            nc.sync.dma_start(qkvw[:], qkvw_d.rearrange("t p o -> p t o"))
            projw = pp.tile([P, CCH, C], BF16, tag="projw")
            nc.sync.dma_start(projw[:], projw_d.rearrange("t p o -> p t o"))
            qkb = pp.tile([P, 4], F32, tag="qkb")
            nc.sync.dma_start(qkb[:], qkbias_d.rearrange("j p one -> p (j one)"))
            projb = pp.tile([P, CCH], F32, tag="projb")
            nc.sync.dma_start(projb[:], projb_d.rearrange("t p one -> p (t one)"))
            gnsc = pp.tile([P, CCH], F32, tag="gnsc")
            nc.sync.dma_start(gnsc[:], gnsc_d.rearrange("t p one -> p (t one)"))
            gnbi = pp.tile([P, CCH], F32, tag="gnbi")
            nc.sync.dma_start(gnbi[:], gnbi_d.rearrange("t p one -> p (t one)"))
            gnind = pp.tile([P, CCH, P], F32, tag="gnind")
            nc.sync.dma_start(gnind[:], gnind_d.rearrange("t p g -> p t g"))
            gnind2 = pp.tile([P, CCH, P], F32, tag="gnind2")
            nc.sync.dma_start(gnind2[:], gnind2_d.rearrange("t g c -> g t c"))
            # V bias broadcast across partitions (DMA with partition-stride 0)
            vbias = pp.tile([P, C], F32, tag="vbias")
            nc.sync.dma_start(vbias[:], vbias_d[None, :].to_broadcast([P, C]))
            # ones for the denominator matmul (fp8, DR: [K=128, 2, M=128])
            ones_f8 = pp.tile([P, 2, P], FP8, tag="ones_f8")
            nc.vector.memset(ones_f8[:], 1.0)
            # single-partition ones column for the V-bias broadcast matmul
            onescol = pp.tile([1, P], F32, tag="onescol")
            nc.vector.memset(onescol[:], 1.0)

            # ---------------- GN stats (read the fp8 x) ----------
            # bn_stats runs on the fp8 copy of x (quantization noise on the
            # mean/var of 128k samples is ~1e-4 relative -- irrelevant), so
            # the f32 x needed only for the residual can stream in lazily
            # during attention.
            stats = pp.tile([P, CCH, 2], F32, tag="stats")
            for t in range(CCH):
                bn6 = wp.tile([P, n_tok // 512, 6], F32, tag="bn6")
                xv = x_f8[:, t].rearrange("p (a b) -> p a b", b=512)
                for a in range(n_tok // 512):
                    nc.vector.bn_stats(bn6[:, a], xv[:, a])
                nc.vector.bn_aggr(stats[:, t], bn6[:])
                # stats col1 := mean^2 + var = E[x^2] (col0 stays mean)
                nc.vector.scalar_tensor_tensor(
                    out=stats[:, t, 1:2],
                    in0=stats[:, t, 0:1],
                    scalar=stats[:, t, 0:1],
                    in1=stats[:, t, 1:2],
                    op0=ALU.mult,
                    op1=ALU.add,
                )
            # f32 x for the residual: queued after everything above, consumed
            # from finalize() well into the attention phase.
            x_sb = pp.tile([P, CCH, n_tok], F32, tag="x_sb")
            for t in range(CCH):
                for pc in range(XPC):
                    xs = slice(pc * (n_tok // XPC), (pc + 1) * (n_tok // XPC))
                    nc.sync.dma_start(x_sb[:, t, xs], x_d[t * P:(t + 1) * P, xs])

            # group aggregation: gagg[g, j] = sum_{c in group g} stats[c, j]
            gagg_ps = psp.tile([P, QT], F32, tag="p", name="gagg_ps")
            for t in range(CCH):
                nc.tensor.matmul(
                    gagg_ps[:, :2],
                    gnind[:, t],
                    stats[:, t],
                    start=(t == 0),
                    stop=(t == CCH - 1),
                )
            # per-group a = rstd, b = -mean * rstd
            gab = pp.tile([P, 2], F32, tag="gab")
            nc.vector.memset(gab[:], 0.0)
            gmean = wp.tile([P, 1], F32, tag="gmean")
            gtmp = wp.tile([P, 1], F32, tag="gtmp")
            nc.vector.tensor_scalar_mul(gmean[:GROUPS], gagg_ps[:GROUPS, 0:1], 1.0 / GSZ)
            nc.vector.tensor_scalar_mul(gtmp[:GROUPS], gagg_ps[:GROUPS, 1:2], 1.0 / GSZ)
            # gtmp := mean^2 - E[x^2] = -var
            nc.vector.scalar_tensor_tensor(
                out=gtmp[:GROUPS],
                in0=gmean[:GROUPS],
                scalar=gmean[:GROUPS],
                in1=gtmp[:GROUPS],
                op0=ALU.mult,
                op1=ALU.subtract,
            )
            # std = sqrt(-1 * gtmp + eps)
            epsb = wp.tile([P, 1], F32, tag="epsb")
            nc.vector.memset(epsb[:], EPS)
            nc.scalar.activation(gtmp[:GROUPS], gtmp[:GROUPS], AF.Sqrt,
                                 bias=epsb[:GROUPS], scale=-1.0)
            nc.vector.reciprocal(gab[:GROUPS, 0:1], gtmp[:GROUPS])  # a = rstd
            nc.vector.tensor_mul(gtmp[:GROUPS], gmean[:GROUPS], gab[:GROUPS, 0:1])
            nc.vector.tensor_scalar_mul(gab[:GROUPS, 1:2], gtmp[:GROUPS], -1.0)

            # broadcast (a, b) to channels; fold GN into the fp8 weights:
            #   m_c = rstd_g * gamma_c ; a_c = (-mean*rstd)*gamma_c + beta_c
            #   w_eff[c, o] = qkvw[c, o] * m_c        (qkvw = 32*W^T)
            #   a2_c = 32 * a_c / m_c   (fp8; a-term via w_eff @ a2 / 1024)
            w_eff = pp.tile([P, CCH, 3 * C], FP8, tag="w_eff")
            a_col = pp.tile([P, CCH, 1], FP8, tag="a_col")
            chms = []
            for t in range(CCH):
                chab_ps = psp.tile([P, QT], F32, tag="p", name=f"chab_ps{t}")
                nc.tensor.matmul(chab_ps[:, :2], gnind2[:, t], gab[:],
                                 start=True, stop=True)
                chm = pp.tile([P, 1], F32, tag=f"chm{t}", name=f"chm{t}")
                cha = pp.tile([P, 1], F32, tag=f"cha{t}", name=f"cha{t}")
                nc.vector.tensor_mul(chm[:], chab_ps[:, 0:1], gnsc[:, t, None])
                nc.vector.scalar_tensor_tensor(
                    out=cha[:],
                    in0=chab_ps[:, 1:2],
                    scalar=gnsc[:, t, None],
                    in1=gnbi[:, t, None],
                    op0=ALU.mult,
                    op1=ALU.add,
                )
                chms.append(chm)
                nc.vector.tensor_scalar_mul(w_eff[:, t], qkvw[:, t], chm[:])
                # a2 = 32 * cha / chm  (fp8)
                rchm = wp.tile([P, 1], F32, tag=f"rchm{t}", name=f"rchm{t}")
                nc.vector.reciprocal(rchm[:], chm[:])
                nc.vector.scalar_tensor_tensor(
                    out=a_col[:, t],
                    in0=cha[:],
                    scalar=WS,
                    in1=rchm[:],
                    op0=ALU.mult,
                    op1=ALU.mult,
                )

            # ---- bias vectors: Wa terms via tiny matmuls ----
            # Q/K: qkb_eff[d, j] = qkb[d, j] + (W@a)[j*128+d] / 1024
            qkb_eff = pp.tile([P, 4], F32, tag="qkb_eff")
            for j in range(4):
                wa_ps = psp.tile([P, QT], F32, tag="p", name=f"wa_ps{j}")
                for t in range(CCH):
                    nc.tensor.matmul(
                        wa_ps[:, 0:1],
                        w_eff[:, t, j * P:(j + 1) * P],
                        a_col[:, t],
                        start=(t == 0),
                        stop=(t == CCH - 1),
                    )
                nc.vector.scalar_tensor_tensor(
                    out=qkb_eff[:, j, None],
                    in0=wa_ps[:, 0:1],
                    scalar=1.0 / (WS * WS),
                    in1=qkb[:, j, None],
                    op0=ALU.mult,
                    op1=ALU.add,
                )
            # V: vb_eff[*, d] = vbias[d] + (Wv@a)[d] / 1024, broadcast to all
            # partitions via a ones-column fp32 matmul.
            vr_ps = psp.tile([P, QT], F32, tag="p", name="vr_ps")
            for t in range(CCH):
                nc.tensor.matmul(
                    vr_ps[:1, :C],
                    a_col[:, t],
                    w_eff[:, t, 2 * C:3 * C],
                    start=(t == 0),
                    stop=(t == CCH - 1),
                )
            vrow = pp.tile([1, C], F32, tag="vrow")
            nc.vector.tensor_copy(vrow[:], vr_ps[:1, :C])
            vb_ps = psp.tile([P, QT], F32, tag="p", name="vb_ps")
            nc.tensor.matmul(vb_ps[:, :C], onescol[:], vrow[:],
                             start=True, stop=True)
            vb_eff = pp.tile([P, C], F32, tag="vb_eff")
            nc.vector.scalar_tensor_tensor(
                out=vb_eff[:],
                in0=vb_ps[:, :C],
                scalar=1.0 / (WS * WS),
                in1=vbias[:],
                op0=ALU.mult,
                op1=ALU.add,
            )

            # ---------------- QKV ----------------
            # Q, K in (d, n) fp8; V token-major fp8. All matmuls fp8 DR
            # (K=256 contraction in one pass). Only the blocks the first
            # attention steps need are emitted up front (copies on ACT,
            # which is otherwise idle before the first exp); the rest are
            # deadline-scheduled INTO the attention loop with copies on the
            # DVE, which is idle during attention.
            qk = pp.tile([P, 4, n_tok], FP8, tag="qk")
            v_sb = pp.tile([P, NKB, C], FP8, tag="v_sb")

            def emit_qk_half(j, h, engine):
                # one [P, 512] half-block of Q (j<2) or K (j>=2). Upfront
                # (pre-attention) halves may borrow the idle "o" tag; once
                # attention runs, the o banks belong to the PV/den tile for
                # 16-step stretches, so in-loop halves use "p" only.
                ns = slice(h * QT, (h + 1) * QT)
                qp = psp.tile([P, QT], F32,
                              tag=("o" if (h == 0 and j % 2) else "p"),
                              name=f"qp{j}_{h}")
                nc.tensor.matmul(
                    qp[:],
                    w_eff[:, :, j * P:(j + 1) * P],
                    x_f8[:, :, ns],
                    start=True,
                    stop=True,
                    perf_mode=DR,
                )
                if engine == "act":
                    nc.scalar.activation(
                        qk[:, j, ns], qp[:],
                        AF.Identity,
                        bias=qkb_eff[:, j, None],
                        scale=INV_WS,
                    )
                else:
                    nc.vector.tensor_scalar(
                        out=qk[:, j, ns],
                        in0=qp[:],
                        scalar1=INV_WS,
                        scalar2=qkb_eff[:, j, None],
                        op0=ALU.mult,
                        op1=ALU.add,
                    )

            def emit_v_block(tp, tag="s"):
                # two 128-token blocks of V: out [tok, 2, C]; copy on DVE
                vp = psp.tile([P, 2, C], F32, tag=tag, bufs=(2 if tag == "s" else 1),
                              name=f"vp{tp}")
                for k2 in range(2):
                    tb = 2 * tp + k2
                    nc.tensor.matmul(
                        vp[:, k2],
                        x_f8[:, :, tb * P:(tb + 1) * P],
                        w_eff[:, :, 2 * C:3 * C],
                        start=True,
                        stop=True,
                        perf_mode=DR,
                    )
                nc.vector.scalar_tensor_tensor(
                    out=v_sb[:, 2 * tp:2 * tp + 2],
                    in0=vp[:],
                    scalar=INV_WS,
                    in1=vb_eff[:, None, :].to_broadcast([P, 2, C]),
                    op0=ALU.mult,
                    op1=ALU.add,
                )

            # upfront: exactly what steps 0-2 consume
            for j in range(4):
                emit_qk_half(j, 0, "act")      # Q qt0 + K kb0-3
            for tp in range(6):
                emit_v_block(tp)

            # deadline schedule for the rest, emitted inside the g-loop
            pending = {}

            def sched(step, fn):
                pending.setdefault(step, []).append(fn)

            for tp in range(6, NKB // 2):
                sched(max(0, tp - 4), lambda tp=tp: emit_v_block(tp, tag="p"))
            for h in range(1, 8):
                for j in (2, 3):               # K half h covers kb 4h..4h+3
                    sched(max(0, 2 * h - 4),
                          lambda j=j, h=h: emit_qk_half(j, h, "dve"))
            for h in range(1, 8):
                for j in (0, 1):               # Q half h needed at qt == h
                    sched(16 * h - 12 + j,
                          lambda j=j, h=h: emit_qk_half(j, h, "dve"))

            # ---------------- attention ----------------
            # global steps g = qt*NKP + i ; per step:
            #   scores(g):  2 DR matmuls -> s_ps [P, 2(kb), 512]
            #   exp(g):     1 ACT instr [P, 1024] -> pt fp8
            #   pv_den(g-2): 2 PV DR matmuls + 1 ones-DR matmul into
            #                o tile [P, 3, 512] (ch0, ch1, den)
            # PV runs TWO steps behind scores so the PE never waits on the
            # scores->exp->pt chain (exp latency > PE slack per step).
            # finalize is split: rec/obs (DVE) emit at (qt, 2) BEFORE
            # pv_den(qt, 0) so the o-tile reuse is ordered; proj/res emit
            # at (qt, 3).
            o_tiles = {}
            pt_tiles = {}

            def emit_scores_exp(g):
                qt, i = divmod(g, NKP)
                qs = slice(qt * QT, (qt + 1) * QT)
                s_ps = psp.tile([P, 2, QT], F32, tag="s", bufs=2,
                                name=f"s_{g}")
                for k2 in range(2):
                    kb = 2 * i + k2
                    nc.tensor.matmul(
                        s_ps[:, k2],
                        qk[:, 2:4, kb * P:(kb + 1) * P],
                        qk[:, 0:2, qs],
                        start=True,
                        stop=True,
                        perf_mode=DR,
                    )
                pt = wp.tile([P, 2, QT], FP8, tag="pt", bufs=6, name=f"pt_{g}")
                nc.scalar.activation(
                    pt.rearrange("p a b -> p (a b)"),
                    s_ps.rearrange("p a b -> p (a b)"),
                    AF.Exp, scale=ATT_SCALE)
                pt_tiles[g] = pt

            def emit_pv_den(g):
                qt, i = divmod(g, NKP)
                if i == 0:
                    o_tiles[qt] = psp.tile([P, 3, QT], F32, tag="o",
                                           name=f"o_{qt}")
                o = o_tiles[qt]
                pt = pt_tiles.pop(g)
                for ch in range(2):
                    nc.tensor.matmul(
                        o[:, ch],
                        v_sb[:, 2 * i:2 * i + 2, ch * P:(ch + 1) * P],
                        pt[:],
                        start=(i == 0),
                        stop=(i == NKP - 1),
                        perf_mode=DR,
                    )
                nc.tensor.matmul(
                    o[:, 2],
                    ones_f8[:],
                    pt[:],
                    start=(i == 0),
                    stop=(i == NKP - 1),
                    perf_mode=DR,
                )

            def finalize_a(qt):
                # denominator reciprocal + normalize (DVE); frees the o tile
                o = o_tiles.pop(qt)
                rec = wp.tile([P, QT], F32, tag="rec", bufs=2, name=f"rec{qt}")
                nc.vector.reciprocal_approx_fast(rec[:], o[:, 2])
                obs = wp.tile([P, 2, QT], BF16, tag="obs", bufs=2,
                              name=f"obs{qt}")
                for ch in range(2):
                    nc.vector.tensor_mul(obs[:, ch], o[:, ch], rec[:])
                return obs

            def finalize_b(qt, obs):
                qs = slice(qt * QT, (qt + 1) * QT)
                for t in range(CCH):
                    p_ps = psp.tile([P, QT], F32, tag="p", name=f"pp_{qt}_{t}")
                    nc.tensor.matmul(p_ps[:],
                                     projw[:, 0, t * P:(t + 1) * P],
                                     obs[:, 0], start=True, stop=False)
                    nc.tensor.matmul(p_ps[:],
                                     projw[:, 1, t * P:(t + 1) * P],
                                     obs[:, 1], start=False, stop=True)
                    res = wp.tile([P, QT], F32, tag="res", bufs=4,
                                  name=f"res{qt}_{t}")
                    nc.vector.scalar_tensor_tensor(
                        out=res[:],
                        in0=p_ps[:],
                        scalar=projb[:, t, None],
                        in1=x_sb[:, t, qs],
                        op0=ALU.add,
                        op1=ALU.add,
                    )
                    nc.sync.dma_start(out_d[t * P:(t + 1) * P, qs], res[:])

            NST = NQ * NKP
            obs_pending = None
            for g in range(NST):
                qt, i = divmod(g, NKP)
                emit_scores_exp(g)
                for fn in pending.pop(g, ()):
                    fn()
                if qt > 0 and i == 2:
                    obs_pending = (qt - 1, finalize_a(qt - 1))
                if g > 1:
                    emit_pv_den(g - 2)
                if qt > 0 and i == 3:
                    finalize_b(*obs_pending)
                    obs_pending = None
            emit_pv_den(NST - 2)
            emit_pv_den(NST - 1)
            finalize_b(NQ - 1, finalize_a(NQ - 1))

    nc.finalize()
    return nc


# ---------------------------------------------------------------------------
# host side
# ---------------------------------------------------------------------------

def _prep_core_inputs(inputs, n_tok=H * W):
    """Build the per-core in_maps (shared weight tensors + per-core x)."""
    CCH = C // P
    f32 = np.float32
    bf16 = ml_dtypes.bfloat16
    fp8 = mybir.dt.np(FP8)

    x = np.asarray(inputs["x"], f32).reshape(B, C, n_tok)
    gn_scale = np.asarray(inputs["gn_scale"], f32)
    gn_bias = np.asarray(inputs["gn_bias"], f32)
    qkv_w = np.asarray(inputs["qkv_w"], f32)
    qkv_b = np.asarray(inputs["qkv_b"], f32)
    proj_w = np.asarray(inputs["proj_w"], f32)
    proj_b = np.asarray(inputs["proj_b"], f32)

    qkv_wt = (np.ascontiguousarray(qkv_w.T) * WS).reshape(CCH, P, 3 * C).astype(fp8)
    qk_bias = qkv_b[:2 * C].reshape(4, P, 1).astype(f32).copy()
    v_bias = qkv_b[2 * C:].astype(f32)
    proj_wt = np.ascontiguousarray(proj_w.T).reshape(CCH, P, C).astype(bf16)
    proj_bt = proj_b.reshape(CCH, P, 1).astype(f32)
    gn_sc = gn_scale.reshape(CCH, P, 1).astype(f32)
    gn_bi = gn_bias.reshape(CCH, P, 1).astype(f32)

    ch = np.arange(C)
    gn_ind = np.zeros((CCH, P, P), f32)
    gn_ind[ch // P, ch % P, ch // (C // GROUPS)] = 1.0
    gn_ind2 = np.zeros((CCH, P, P), f32)
    for t in range(CCH):
        gn_ind2[t, :GROUPS, :] = gn_ind[t, :, :GROUPS].T

    shared = {
        "qkv_wt": qkv_wt,
        "qk_bias": qk_bias,
        "v_bias": v_bias,
        "proj_wt": proj_wt,
        "proj_b": proj_bt,
        "gn_sc": gn_sc,
        "gn_bi": gn_bi,
        "gn_ind": gn_ind,
        "gn_ind2": gn_ind2,
    }
    x_f8 = x.reshape(B, CCH, P, n_tok).astype(fp8)
    return [
        dict(shared, x=np.ascontiguousarray(x[i]), x_f8=np.ascontiguousarray(x_f8[i]))
        for i in range(B)
    ]


_NC_CACHE = {}
LAST_RESULT = None  # BassKernelResults of the most recent run (for test.py)


def _get_nc():
    if "nc" not in _NC_CACHE:
        _NC_CACHE["nc"] = build_nc()
    return _NC_CACHE["nc"]


def kernel(**inputs) -> np.ndarray:
    global LAST_RESULT
    from concourse.bass_utils import run_bass_kernel_spmd

    nc = _get_nc()
    in_maps = _prep_core_inputs(inputs)
    res = run_bass_kernel_spmd(nc, in_maps, list(range(N_CORES)))
    LAST_RESULT = res
    out = np.stack([np.asarray(res.results[i]["out"]) for i in range(B)])
    return out.reshape(B, C, H, W).astype(np.float32)


# revision 17
# speedup vs baseline: 1.1488x; 1.0179x over previous
"""Trainium2 Bass kernel for nn_AttentionBlock (GroupNorm + single-head
self-attention + proj + residual), data-parallel over batch on 8 cores.

Contract: kernel(**inputs) takes the FULL unsharded inputs
  x (8, 256, 64, 64) f32, gn_scale (256,), gn_bias (256,),
  qkv_w (768, 256), qkv_b (768,), proj_w (256, 256), proj_b (256,)
and returns the FULL output (8, 256, 64, 64) f32.

v2 design (from the v1 NTFF trace: PE 90% busy on matmuls, ACT co-bound
on exp, DVE saturated by denominator accumulation):
  - GroupNorm folded into the QKV weights: w_eff[c,o] = 32*W[o,c]*m_c on
    device (m_c = rstd*gamma per channel), so no xn tensor is ever
    materialized. The additive GN term (a_c) becomes per-output biases
    via tiny matmuls (W@a). x is shipped from host in BOTH f32 (GN stats
    + residual) and fp8 (QKV matmul operand).
  - QKV/scores/PV all fp8 DoubleRow (K=256 in one pass).
  - Softmax denominator on the PE: a ones-lhsT DR matmul per key-block
    pair accumulates den[q] into the same PSUM tile group as the PV
    output (tile [P, 3, 512]: ch0, ch1, den) -> zero DVE work in the
    steady loop.
  - Steady state per 512-q-tile step: PE 5 matmuls (2 scores, 2 PV,
    1 den) ~1.1us; ACT one 1024-wide exp ~1.1us. PV/den run one step
    behind scores so ACT never waits on PE.
  - PSUM banks: scores 2x[P,2,512]=4, out+den [P,3,512]=3, proj 1 = 8.
"""

import os
import sys

import numpy as np

for _p in (
    "/opt/trn_rl_repo",
    "/root/.axon_site",
    "/root/.axon_site/_ro/trn_rl_repo",
    "/root/.axon_site/_ro/pypackages",
):
    if os.path.isdir(_p) and _p not in sys.path:
        sys.path.append(_p)

import ml_dtypes  # noqa: E402

import concourse.bass as bass  # noqa: E402
import concourse.mybir as mybir  # noqa: E402
import concourse.tile as tile  # noqa: E402
from concourse import bacc  # noqa: E402

F32 = mybir.dt.float32
BF16 = mybir.dt.bfloat16
FP8 = mybir.dt.float8e4
AF = mybir.ActivationFunctionType
ALU = mybir.AluOpType
DR = mybir.MatmulPerfMode.DoubleRow

B, C, H, W = 8, 256, 64, 64
GROUPS = 8
EPS = 1e-5
P = 128
N_CORES = 8
ATT_SCALE = float(C) ** -0.5  # 1/16
WS = 32.0                     # host pre-scale on fp8 qkv weights
INV_WS = 1.0 / WS


def build_nc(n_tok=H * W):
    """Build the single-core Bass program (SPMD across 8 cores)."""
    CCH = C // P            # channel chunks (2)
    QT = 512                # q-tile width (one PSUM bank of f32)
    NQ = n_tok // QT        # number of q tiles (8)
    NKB = n_tok // P        # number of 128-token key blocks (32)
    NKP = NKB // 2          # key-block pairs per q tile (16)
    GSZ = C // GROUPS       # channels per group (32)

    nc = bacc.Bacc()

    # ---- DRAM I/O (per-core tensors; host shards batch over cores) ----
    x_d = nc.dram_tensor("x", [C, n_tok], F32, kind="ExternalInput")
    xf8_d = nc.dram_tensor("x_f8", [CCH, P, n_tok], FP8, kind="ExternalInput")
    qkvw_d = nc.dram_tensor("qkv_wt", [CCH, P, 3 * C], FP8, kind="ExternalInput")
    qkbias_d = nc.dram_tensor("qk_bias", [4, P, 1], F32, kind="ExternalInput")
    vbias_d = nc.dram_tensor("v_bias", [C], F32, kind="ExternalInput")
    projw_d = nc.dram_tensor("proj_wt", [CCH, P, C], BF16, kind="ExternalInput")
    projb_d = nc.dram_tensor("proj_b", [CCH, P, 1], F32, kind="ExternalInput")
    gnsc_d = nc.dram_tensor("gn_sc", [CCH, P, 1], F32, kind="ExternalInput")
    gnbi_d = nc.dram_tensor("gn_bi", [CCH, P, 1], F32, kind="ExternalInput")
    # group-sum indicator (zero-padded to M=128): ind[t, c, g] = (t*128+c)//32 == g
    gnind_d = nc.dram_tensor("gn_ind", [CCH, P, P], F32, kind="ExternalInput")
    # channel-broadcast indicator, padded to K=128: ind2[t, g, c] nonzero only g<8
    gnind2_d = nc.dram_tensor("gn_ind2", [CCH, P, P], F32, kind="ExternalInput")
    out_d = nc.dram_tensor("out", [C, n_tok], F32, kind="ExternalOutput")

    with tile.TileContext(nc) as tc:
        with (
            tc.tile_pool(name="persist", bufs=1) as pp,
            tc.tile_pool(name="work", bufs=3) as wp,
            tc.tile_pool(name="ps", bufs=1, space="PSUM") as psp,
        ):
            # ---------------- load x_f8 first (feeds GN stats + QKV) ------
            x_f8 = pp.tile([P, CCH, n_tok], FP8, tag="x_f8")
            XPC = 4
            for pc in range(XPC):
                xs = slice(pc * (n_tok // XPC), (pc + 1) * (n_tok // XPC))
                nc.sync.dma_start(
                    x_f8[:, :, xs],
                    xf8_d.rearrange("t p n -> p t n")[:, :, xs],
                )

            # ---------------- load weights / constants ----------------
            qkvw = pp.tile([P, CCH, 3 * C], FP8, tag="qkvw")
            nc.sync.dma_start(qkvw[:], qkvw_d.rearrange("t p o -> p t o"))
            projw = pp.tile([P, CCH, C], BF16, tag="projw")
            nc.sync.dma_start(projw[:], projw_d.rearrange("t p o -> p t o"))
            qkb = pp.tile([P, 4], F32, tag="qkb")
            nc.sync.dma_start(qkb[:], qkbias_d.rearrange("j p one -> p (j one)"))
            projb = pp.tile([P, CCH], F32, tag="projb")
            nc.sync.dma_start(projb[:], projb_d.rearrange("t p one -> p (t one)"))
            gnsc = pp.tile([P, CCH], F32, tag="gnsc")
            nc.sync.dma_start(gnsc[:], gnsc_d.rearrange("t p one -> p (t one)"))
            gnbi = pp.tile([P, CCH], F32, tag="gnbi")
            nc.sync.dma_start(gnbi[:], gnbi_d.rearrange("t p one -> p (t one)"))
            gnind = pp.tile([P, CCH, P], F32, tag="gnind")
            nc.sync.dma_start(gnind[:], gnind_d.rearrange("t p g -> p t g"))
            gnind2 = pp.tile([P, CCH, P], F32, tag="gnind2")
            nc.sync.dma_start(gnind2[:], gnind2_d.rearrange("t g c -> g t c"))
            # V bias broadcast across partitions (DMA with partition-stride 0)
            vbias = pp.tile([P, C], F32, tag="vbias")
            nc.sync.dma_start(vbias[:], vbias_d[None, :].to_broadcast([P, C]))
            # ones for the denominator matmul (fp8, DR: [K=128, 2, M=128])
            ones_f8 = pp.tile([P, 2, P], FP8, tag="ones_f8")
            nc.vector.memset(ones_f8[:], 1.0)
            # single-partition ones column for the V-bias broadcast matmul
            onescol = pp.tile([1, P], F32, tag="onescol")
            nc.vector.memset(onescol[:], 1.0)

            # ---------------- GN stats (read the fp8 x) ----------
            # bn_stats runs on the fp8 copy of x (quantization noise on the
            # mean/var of 128k samples is ~1e-4 relative -- irrelevant), so
            # the f32 x needed only for the residual can stream in lazily
            # during attention.
            stats = pp.tile([P, CCH, 2], F32, tag="stats")
            for t in range(CCH):
                bn6 = wp.tile([P, n_tok // 512, 6], F32, tag="bn6")
                xv = x_f8[:, t].rearrange("p (a b) -> p a b", b=512)
                for a in range(n_tok // 512):
                    nc.vector.bn_stats(bn6[:, a], xv[:, a])
                nc.vector.bn_aggr(stats[:, t], bn6[:])
                # stats col1 := mean^2 + var = E[x^2] (col0 stays mean)
                nc.vector.scalar_tensor_tensor(
                    out=stats[:, t, 1:2],
                    in0=stats[:, t, 0:1],
                    scalar=stats[:, t, 0:1],
                    in1=stats[:, t, 1:2],
                    op0=ALU.mult,
                    op1=ALU.add,
                )
            # f32 x for the residual: queued after everything above, consumed
            # from finalize() well into the attention phase.
            x_sb = pp.tile([P, CCH, n_tok], F32, tag="x_sb")
            for t in range(CCH):
                for pc in range(XPC):
                    xs = slice(pc * (n_tok // XPC), (pc + 1) * (n_tok // XPC))
                    nc.sync.dma_start(x_sb[:, t, xs], x_d[t * P:(t + 1) * P, xs])

            # group aggregation: gagg[g, j] = sum_{c in group g} stats[c, j]
            gagg_ps = psp.tile([P, QT], F32, tag="p", name="gagg_ps")
            for t in range(CCH):
                nc.tensor.matmul(
                    gagg_ps[:, :2],
                    gnind[:, t],
                    stats[:, t],
                    start=(t == 0),
                    stop=(t == CCH - 1),
                )
            # per-group a = rstd, b = -mean * rstd
            gab = pp.tile([P, 2], F32, tag="gab")
            nc.vector.memset(gab[:], 0.0)
            gmean = wp.tile([P, 1], F32, tag="gmean")
            gtmp = wp.tile([P, 1], F32, tag="gtmp")
            nc.vector.tensor_scalar_mul(gmean[:GROUPS], gagg_ps[:GROUPS, 0:1], 1.0 / GSZ)
            nc.vector.tensor_scalar_mul(gtmp[:GROUPS], gagg_ps[:GROUPS, 1:2], 1.0 / GSZ)
            # gtmp := mean^2 - E[x^2] = -var
            nc.vector.scalar_tensor_tensor(
                out=gtmp[:GROUPS],
                in0=gmean[:GROUPS],
                scalar=gmean[:GROUPS],
                in1=gtmp[:GROUPS],
                op0=ALU.mult,
                op1=ALU.subtract,
            )
            # std = sqrt(-1 * gtmp + eps)
            epsb = wp.tile([P, 1], F32, tag="epsb")
            nc.vector.memset(epsb[:], EPS)
            nc.scalar.activation(gtmp[:GROUPS], gtmp[:GROUPS], AF.Sqrt,
                                 bias=epsb[:GROUPS], scale=-1.0)
            nc.vector.reciprocal(gab[:GROUPS, 0:1], gtmp[:GROUPS])  # a = rstd
            nc.vector.tensor_mul(gtmp[:GROUPS], gmean[:GROUPS], gab[:GROUPS, 0:1])
            nc.vector.tensor_scalar_mul(gab[:GROUPS, 1:2], gtmp[:GROUPS], -1.0)

            # broadcast (a, b) to channels; fold GN into the fp8 weights:
            #   m_c = rstd_g * gamma_c ; a_c = (-mean*rstd)*gamma_c + beta_c
            #   w_eff[c, o] = qkvw[c, o] * m_c        (qkvw = 32*W^T)
            #   a2_c = 32 * a_c / m_c   (fp8; a-term via w_eff @ a2 / 1024)
            w_eff = pp.tile([P, CCH, 3 * C], FP8, tag="w_eff")
            a_col = pp.tile([P, CCH, 1], FP8, tag="a_col")
            chms = []
            for t in range(CCH):
                chab_ps = psp.tile([P, QT], F32, tag="p", name=f"chab_ps{t}")
                nc.tensor.matmul(chab_ps[:, :2], gnind2[:, t], gab[:],
                                 start=True, stop=True)
                chm = pp.tile([P, 1], F32, tag=f"chm{t}", name=f"chm{t}")
                cha = pp.tile([P, 1], F32, tag=f"cha{t}", name=f"cha{t}")
                nc.vector.tensor_mul(chm[:], chab_ps[:, 0:1], gnsc[:, t, None])
                nc.vector.scalar_tensor_tensor(
                    out=cha[:],
                    in0=chab_ps[:, 1:2],
                    scalar=gnsc[:, t, None],
                    in1=gnbi[:, t, None],
                    op0=ALU.mult,
                    op1=ALU.add,
                )
                chms.append(chm)
                nc.vector.tensor_scalar_mul(w_eff[:, t], qkvw[:, t], chm[:])
                # a2 = 32 * cha / chm  (fp8)
                rchm = wp.tile([P, 1], F32, tag=f"rchm{t}", name=f"rchm{t}")
                nc.vector.reciprocal(rchm[:], chm[:])
                nc.vector.scalar_tensor_tensor(
                    out=a_col[:, t],
                    in0=cha[:],
                    scalar=WS,
                    in1=rchm[:],
                    op0=ALU.mult,
                    op1=ALU.mult,
                )

            # ---- bias vectors: Wa terms via tiny matmuls ----
            # Q/K: qkb_eff[d, j] = qkb[d, j] + (W@a)[j*128+d] / 1024
            qkb_eff = pp.tile([P, 4], F32, tag="qkb_eff")
            for j in range(4):
                wa_ps = psp.tile([P, QT], F32, tag="p", name=f"wa_ps{j}")
                for t in range(CCH):
                    nc.tensor.matmul(
                        wa_ps[:, 0:1],
                        w_eff[:, t, j * P:(j + 1) * P],
                        a_col[:, t],
                        start=(t == 0),
                        stop=(t == CCH - 1),
                    )
                nc.vector.scalar_tensor_tensor(
                    out=qkb_eff[:, j, None],
                    in0=wa_ps[:, 0:1],
                    scalar=1.0 / (WS * WS),
                    in1=qkb[:, j, None],
                    op0=ALU.mult,
                    op1=ALU.add,
                )
            # V: vb_eff[*, d] = vbias[d] + (Wv@a)[d] / 1024, broadcast to all
            # partitions via a ones-column fp32 matmul.
            vr_ps = psp.tile([P, QT], F32, tag="p", name="vr_ps")
            for t in range(CCH):
                nc.tensor.matmul(
                    vr_ps[:1, :C],
                    a_col[:, t],
                    w_eff[:, t, 2 * C:3 * C],
                    start=(t == 0),
                    stop=(t == CCH - 1),
                )
            vrow = pp.tile([1, C], F32, tag="vrow")
            nc.vector.tensor_copy(vrow[:], vr_ps[:1, :C])
            vb_ps = psp.tile([P, QT], F32, tag="p", name="vb_ps")
            nc.tensor.matmul(vb_ps[:, :C], onescol[:], vrow[:],
                             start=True, stop=True)
            vb_eff = pp.tile([P, C], F32, tag="vb_eff")
            nc.vector.scalar_tensor_tensor(
                out=vb_eff[:],
                in0=vb_ps[:, :C],
                scalar=1.0 / (WS * WS),
                in1=vbias[:],
                op0=ALU.mult,
                op1=ALU.add,
            )

            # ---------------- QKV ----------------
            # Q, K in (d, n) fp8; V token-major fp8. All matmuls fp8 DR
            # (K=256 contraction in one pass). Only the blocks the first
            # attention steps need are emitted up front (copies on ACT,
            # which is otherwise idle before the first exp); the rest are
            # deadline-scheduled INTO the attention loop with copies on the
            # DVE, which is idle during attention.
            qk = pp.tile([P, 4, n_tok], FP8, tag="qk")
            v_sb = pp.tile([P, NKB, C], FP8, tag="v_sb")

            import itertools
            _tag_cycle = itertools.cycle([("s", 2), ("o", 1), ("s", 2), ("p", 1)])

            def emit_qk_half(j, h, engine, tag=None):
                # one [P, 512] half-block of Q (j<2) or K (j>=2). Upfront
                # (pre-attention) tiles cycle across all psum tags; in-loop
                # Q halves ride the "s" rotation (transient, DVE-copied).
                ns = slice(h * QT, (h + 1) * QT)
                tg, bf = tag if tag else ("s", 2)
                qp = psp.tile([P, QT], F32, tag=tg, bufs=bf,
                              name=f"qp{j}_{h}")
                nc.tensor.matmul(
                    qp[:],
                    w_eff[:, :, j * P:(j + 1) * P],
                    x_f8[:, :, ns],
                    start=True,
                    stop=True,
                    perf_mode=DR,
                )
                if engine == "act":
                    nc.scalar.activation(
                        qk[:, j, ns], qp[:],
                        AF.Identity,
                        bias=qkb_eff[:, j, None],
                        scale=INV_WS,
                    )
                else:
                    nc.vector.tensor_scalar(
                        out=qk[:, j, ns],
                        in0=qp[:],
                        scalar1=INV_WS,
                        scalar2=qkb_eff[:, j, None],
                        op0=ALU.mult,
                        op1=ALU.add,
                    )

            def emit_v_block(tp, engine="dve", tag=None):
                # two 128-token blocks of V: out [tok, 2, C]
                tg, bf = tag if tag else ("s", 2)
                vp = psp.tile([P, 2, C], F32, tag=tg, bufs=bf, name=f"vp{tp}")
                for k2 in range(2):
                    tb = 2 * tp + k2
                    nc.tensor.matmul(
                        vp[:, k2],
                        x_f8[:, :, tb * P:(tb + 1) * P],
                        w_eff[:, :, 2 * C:3 * C],
                        start=True,
                        stop=True,
                        perf_mode=DR,
                    )
                nc.vector.scalar_tensor_tensor(
                    out=v_sb[:, 2 * tp:2 * tp + 2],
                    in0=vp[:],
                    scalar=INV_WS,
                    in1=vb_eff[:, None, :].to_broadcast([P, 2, C]),
                    op0=ALU.mult,
                    op1=ALU.add,
                )

            # All of V and K plus Q half 0 run before the attention loop,
            # PSUM tiles cycling over every tag so no single bank chain
            # serializes; copies balanced across ACT and DVE.
            for j in (2, 3, 0, 1):
                emit_qk_half(j, 0, "act", next(_tag_cycle))
            _kq = ([(2, h) for h in range(1, 8)] + [(3, h) for h in range(1, 8)])
            for r in range(16):
                emit_v_block(r, "dve", next(_tag_cycle))
                if r < 14:
                    j, h = _kq[r]
                    emit_qk_half(j, h, "act" if h < 4 else "dve",
                                 next(_tag_cycle))
            for j, h in _kq[14:]:
                emit_qk_half(j, h, "dve", next(_tag_cycle))

            # Q halves 1-7 are deadline-scheduled into the attention loop
            # (needed at q-tile h = step 16h); copies on the idle DVE.
            pending = {}

            def sched(step, fn):
                pending.setdefault(step, []).append(fn)

            for h in range(1, 8):
                for j in (0, 1):
                    sched(16 * h - 12 + j,
                          lambda j=j, h=h: emit_qk_half(j, h, "dve"))

            # ---------------- attention ----------------
            # global steps g = qt*NKP + i ; per step:
            #   scores(g):  2 DR matmuls -> s_ps [P, 2(kb), 512]
            #   exp(g):     1 ACT instr [P, 1024] -> pt fp8
            #   pv_den(g-2): 2 PV DR matmuls + 1 ones-DR matmul into
            #                o tile [P, 3, 512] (ch0, ch1, den)
            # PV runs TWO steps behind scores so the PE never waits on the
            # scores->exp->pt chain (exp latency > PE slack per step).
            # finalize is split: rec/obs (DVE) emit at (qt, 2) BEFORE
            # pv_den(qt, 0) so the o-tile reuse is ordered; proj/res emit
            # at (qt, 3).
            o_tiles = {}
            pt_tiles = {}

            def emit_scores_exp(g):
                qt, i = divmod(g, NKP)
                qs = slice(qt * QT, (qt + 1) * QT)
                s_ps = psp.tile([P, 2, QT], F32, tag="s", bufs=2,
                                name=f"s_{g}")
                for k2 in range(2):
                    kb = 2 * i + k2
                    nc.tensor.matmul(
                        s_ps[:, k2],
                        qk[:, 2:4, kb * P:(kb + 1) * P],
                        qk[:, 0:2, qs],
                        start=True,
                        stop=True,
                        perf_mode=DR,
                    )
                pt = wp.tile([P, 2, QT], FP8, tag="pt", bufs=6, name=f"pt_{g}")
                nc.scalar.activation(
                    pt.rearrange("p a b -> p (a b)"),
                    s_ps.rearrange("p a b -> p (a b)"),
                    AF.Exp, scale=ATT_SCALE)
                pt_tiles[g] = pt

            def emit_pv_den(g):
                qt, i = divmod(g, NKP)
                if i == 0:
                    o_tiles[qt] = psp.tile([P, 3, QT], F32, tag="o",
                                           name=f"o_{qt}")
                o = o_tiles[qt]
                pt = pt_tiles.pop(g)
                for ch in range(2):
                    nc.tensor.matmul(
                        o[:, ch],
                        v_sb[:, 2 * i:2 * i + 2, ch * P:(ch + 1) * P],
                        pt[:],
                        start=(i == 0),
                        stop=(i == NKP - 1),
                        perf_mode=DR,
                    )
                nc.tensor.matmul(
                    o[:, 2],
                    ones_f8[:],
                    pt[:],
                    start=(i == 0),
                    stop=(i == NKP - 1),
                    perf_mode=DR,
                )

            def finalize_a(qt):
                # UN-normalized bf16 copies of the PV output (frees the o
                # tile without waiting on the reciprocal chain) + the
                # denominator reciprocal. Normalization happens after proj:
                # proj is linear, so proj(out/den) == proj(out)/den.
                o = o_tiles.pop(qt)
                obs = wp.tile([P, 2, QT], BF16, tag="obs", bufs=2,
                              name=f"obs{qt}")
                for ch in range(2):
                    nc.vector.tensor_copy(obs[:, ch], o[:, ch])
                rec = wp.tile([P, QT], F32, tag="rec", bufs=2, name=f"rec{qt}")
                nc.vector.reciprocal_approx_fast(rec[:], o[:, 2])
                return obs, rec

            def finalize_b(qt, obs, rec):
                qs = slice(qt * QT, (qt + 1) * QT)
                for t in range(CCH):
                    p_ps = psp.tile([P, QT], F32, tag="p", name=f"pp_{qt}_{t}")
                    nc.tensor.matmul(p_ps[:],
                                     projw[:, 0, t * P:(t + 1) * P],
                                     obs[:, 0], start=True, stop=False)
                    nc.tensor.matmul(p_ps[:],
                                     projw[:, 1, t * P:(t + 1) * P],
                                     obs[:, 1], start=False, stop=True)
                    tmp = wp.tile([P, QT], F32, tag="tmp", bufs=2,
                                  name=f"tmp{qt}_{t}")
                    nc.vector.tensor_mul(tmp[:], p_ps[:], rec[:])
                    res = wp.tile([P, QT], F32, tag="res", bufs=4,
                                  name=f"res{qt}_{t}")
                    nc.vector.scalar_tensor_tensor(
                        out=res[:],
                        in0=tmp[:],
                        scalar=projb[:, t, None],
                        in1=x_sb[:, t, qs],
                        op0=ALU.add,
                        op1=ALU.add,
                    )
                    nc.sync.dma_start(out_d[t * P:(t + 1) * P, qs], res[:])

            NST = NQ * NKP
            obs_pending = None
            for g in range(NST):
                qt, i = divmod(g, NKP)
                emit_scores_exp(g)
                for fn in pending.pop(g, ()):
                    fn()
                if qt > 0 and i == 2:
                    obs_pending = (qt - 1,) + finalize_a(qt - 1)
                if g > 1:
                    emit_pv_den(g - 2)
                if qt > 0 and i == 3:
                    finalize_b(*obs_pending)
                    obs_pending = None
            emit_pv_den(NST - 2)
            emit_pv_den(NST - 1)
            finalize_b(NQ - 1, *finalize_a(NQ - 1))

    nc.finalize()
    return nc


# ---------------------------------------------------------------------------
# host side
# ---------------------------------------------------------------------------

def _prep_core_inputs(inputs, n_tok=H * W):
    """Build the per-core in_maps (shared weight tensors + per-core x)."""
    CCH = C // P
    f32 = np.float32
    bf16 = ml_dtypes.bfloat16
    fp8 = mybir.dt.np(FP8)

    x = np.asarray(inputs["x"], f32).reshape(B, C, n_tok)
    gn_scale = np.asarray(inputs["gn_scale"], f32)
    gn_bias = np.asarray(inputs["gn_bias"], f32)
    qkv_w = np.asarray(inputs["qkv_w"], f32)
    qkv_b = np.asarray(inputs["qkv_b"], f32)
    proj_w = np.asarray(inputs["proj_w"], f32)
    proj_b = np.asarray(inputs["proj_b"], f32)

    qkv_wt = (np.ascontiguousarray(qkv_w.T) * WS).reshape(CCH, P, 3 * C).astype(fp8)
    qk_bias = qkv_b[:2 * C].reshape(4, P, 1).astype(f32).copy()
    v_bias = qkv_b[2 * C:].astype(f32)
    proj_wt = np.ascontiguousarray(proj_w.T).reshape(CCH, P, C).astype(bf16)
    proj_bt = proj_b.reshape(CCH, P, 1).astype(f32)
    gn_sc = gn_scale.reshape(CCH, P, 1).astype(f32)
    gn_bi = gn_bias.reshape(CCH, P, 1).astype(f32)

    ch = np.arange(C)
    gn_ind = np.zeros((CCH, P, P), f32)
    gn_ind[ch // P, ch % P, ch // (C // GROUPS)] = 1.0
    gn_ind2 = np.zeros((CCH, P, P), f32)
    for t in range(CCH):
        gn_ind2[t, :GROUPS, :] = gn_ind[t, :, :GROUPS].T

    shared = {
        "qkv_wt": qkv_wt,
        "qk_bias": qk_bias,
        "v_bias": v_bias,
        "proj_wt": proj_wt,
        "proj_b": proj_bt,
        "gn_sc": gn_sc,
        "gn_bi": gn_bi,
        "gn_ind": gn_ind,
        "gn_ind2": gn_ind2,
    }
    x_f8 = x.reshape(B, CCH, P, n_tok).astype(fp8)
    return [
        dict(shared, x=np.ascontiguousarray(x[i]), x_f8=np.ascontiguousarray(x_f8[i]))
        for i in range(B)
    ]


_NC_CACHE = {}
LAST_RESULT = None  # BassKernelResults of the most recent run (for test.py)


def _get_nc():
    if "nc" not in _NC_CACHE:
        _NC_CACHE["nc"] = build_nc()
    return _NC_CACHE["nc"]


def kernel(**inputs) -> np.ndarray:
    global LAST_RESULT
    from concourse.bass_utils import run_bass_kernel_spmd

    nc = _get_nc()
    in_maps = _prep_core_inputs(inputs)
    res = run_bass_kernel_spmd(nc, in_maps, list(range(N_CORES)))
    LAST_RESULT = res
    out = np.stack([np.asarray(res.results[i]["out"]) for i in range(B)])
    return out.reshape(B, C, H, W).astype(np.float32)
